# revision 36
# baseline (speedup 1.0000x reference)
"""Bass/Tile TRN2 kernel for nn_BlockLearnableCompressionMatrix.

Computes, for x (B=16, 1, n=1024, T=2048) f32 and blocks (m=128, c=8) f32:
    w      = tanh(blocks)                          # (128, 8)
    Ax     = einsum('bmct,mc->bmt', x.reshape(B, 128, 8, T), w)   # (16, 128, 2048)
    A_full = block-diagonal expansion of w         # (128, 1024)

Sharding: pure data parallel over batch B across 8 NeuronCores (2 batches
per core); blocks replicated. Each core computes its Ax shard; A_full is
computed redundantly on every core and taken from core 0.

Per-core schedule (memory-bound; ~358 GB/s/core HBM is the roofline — the
kernel moves 19.4 MB/core, so ~54 us of DMA is the floor; measured
~74-77 us incl. ~12 us of framework preamble/barrier):
  - The whole x shard (16 MB = 128 KB/partition) is RESIDENT in one big
    SBUF tile; the 10 load pieces (small first/last for early compute start
    and a short tail) stream back-to-back on the sync HWDGE ring with no
    buffer-recycle waits, sustaining 300-414 GB/s.
  - Per t-chunk, the segment reduce over c=8 splits across engines:
    ACT computes 4 products p_c = x_c * w[:,c] (per-partition scale) into
    PSUM (keeps SBUF ports free for the DMA write stream); DVE does 4
    fused multiply-adds a_i = x_i*w[:,i] + p_{i+4} and 3 pairwise adds,
    the last written into a per-batch SBUF output tile.
  - HBM writes cost ~1 us per descriptor per SDMA engine, so write
    descriptor count dominates every output: Ax goes out as ONE fat store
    per batch (128 x 8 KB rows), and A_full is built as a 16-partition
    image of 8 [w_m | zeros] periods each, so its body writes with just
    16 x 33 KB descriptors (flat A_full is exactly
    [diag_0 | gap_0 | diag_1 | ... | diag_127]).
"""

import os
import sys
import types

import numpy as np

# ---------------------------------------------------------------------------
# NTFF profile hook injection: the trimmed container's `antenv` package lacks
# `axon_hooks`, which bass_utils needs for trace=True under axon. Register the
# ctypes-based hook from trn_agent_boot so profiling works when requested.
# Harmless (and skipped) when unavailable or when tracing is never requested.
try:
    import antenv
    from trn_agent_boot.trn_boot import _ntff_profile_via_ctypes

    if "antenv.axon_hooks" not in sys.modules:
        _hook = _ntff_profile_via_ctypes("/opt/axon/libaxon_pjrt.so")
        _mod = types.ModuleType("antenv.axon_hooks")
        _mod.get_axon_ntff_profile_hook = lambda: _hook
        sys.modules["antenv.axon_hooks"] = _mod
        antenv.axon_hooks = _mod
except Exception:
    pass

import concourse.bass as bass
import concourse.bacc as bacc
import concourse.mybir as mybir
import concourse.tile as tile
from concourse.bass_utils import run_bass_kernel_spmd

N_CORES = 8
B = 16                 # full batch
B_SH = B // N_CORES    # batches per core
M = 128                # number of blocks == partitions
C = 8                  # block size
N_CH = M * C           # 1024 input channels
T = 2048               # time dim
# Asymmetric t-tiling per batch: small first pieces so compute starts early
# (co-queued transfers round-robin on the SDMA engines, so first-piece
# latency grows with queued bytes), small last pieces for a short tail.
TILES_B0 = [128, 384, 512, 512, 512]
TILES_B1 = [512, 512, 512, 384, 128]

F32 = mybir.dt.float32

# Results of the last run (BassKernelResults), for test harnesses that want
# exec_time_ns from a traced run.
LAST_RESULTS = None

_COMPILED = None  # cached bass module


def _build_module():
    mult = mybir.AluOpType.mult
    add = mybir.AluOpType.add
    nc = bacc.Bacc(
        "TRN2",
        target_bir_lowering=False,
        debug=False,
        enable_asserts=False,
        num_devices=N_CORES,
    )
    x = nc.dram_tensor("x", [B_SH, N_CH, T], F32, kind="ExternalInput").ap()
    blocks = nc.dram_tensor("blocks", [M, C], F32, kind="ExternalInput").ap()
    ax = nc.dram_tensor("ax", [B_SH, M, T], F32, kind="ExternalOutput").ap()
    afull = nc.dram_tensor("afull", [M, N_CH], F32, kind="ExternalOutput").ap()

    # x viewed as (b, m, c, t): channel ch = m*8 + c
    xg = x.rearrange("b (m c) t -> b m c t", c=C)

    with tile.TileContext(nc) as tc:
        with (
            tc.tile_pool(name="consts", bufs=1) as cpool,
            tc.tile_pool(name="xin", bufs=1) as xpool,
            tc.tile_pool(name="prod", bufs=2, space="PSUM") as ppool,
            tc.tile_pool(name="acc", bufs=3) as apool,
            tc.tile_pool(name="out", bufs=2) as opool,
        ):
            # --- weights: w = tanh(blocks). Single direct load: one hop
            # through the flooded SDMA engines beats two (a 1-descriptor
            # flat load + local spread each wait ~4-8 us for engine slots
            # once all 8 cores are streaming x).
            blocks_sb = cpool.tile([M, C], F32)
            nc.scalar.dma_start(blocks_sb[:], blocks[:])
            w_sb = cpool.tile([M, C], F32)
            nc.scalar.activation(
                w_sb[:], blocks_sb[:], mybir.ActivationFunctionType.Tanh
            )

            # --- A_full. Flat layout is [diag_0 (8) | gap_0 (1024) | ...
            # | diag_127 (8)]; build an image where each of 16 partitions
            # holds 8 consecutive [w_m | zeros] periods (8256 elems), so the
            # DRAM write needs only 16 fat descriptors (~33 KB each) instead
            # of 128 -- HBM writes cost ~1 us per descriptor per SDMA engine,
            # so descriptor count is the whole cost of this output.
            img = cpool.tile([16, 8 * 1032], F32)
            nc.gpsimd.memset(img[:], 0.0)
            img_diag = img[:].rearrange("p (k q) -> p k q", q=1032)[:, :, 0:C]
            nc.gpsimd.dma_start(img_diag, w_sb[:])
            body = afull.copy()
            _ap = body.ap
            _ap[0] = [8 * 1032, 15]
            _ap[1] = [1, 8 * 1032]
            body.ap = _ap
            nc.gpsimd.dma_start(body, img[0:15, :])
            tail = afull.copy()
            tail.offset = 15 * 8 * 1032
            _ap = tail.ap
            _ap[0] = [7232, 1]
            _ap[1] = [1, 7232]
            tail.ap = _ap
            nc.gpsimd.dma_start(tail, img[15:16, 0:7232])

            # --- main loop over (batch, t-chunk) against the resident x ---
            xbig = xpool.tile([M, C, B_SH * T], F32)
            for b, widths in ((0, TILES_B0), (1, TILES_B1)):
                out_b = opool.tile([M, T], F32, tag="out")
                t0 = 0
                stored = 0
                for w_t in widths:
                    ts = slice(t0, t0 + w_t)
                    bts = slice(b * T + t0, b * T + t0 + w_t)
                    t0 += w_t
                    xt = xbig[:, :, bts]
                    nc.sync.dma_start(xt, xg[b, :, :, ts])

                    # ACT: products for c = 4..7
                    prods = []
                    for c in range(C // 2, C):
                        p = ppool.tile([M, w_t], F32, tag=f"p{c}")
                        nc.scalar.activation(
                            p[:],
                            xt[:, c, :],
                            mybir.ActivationFunctionType.Copy,
                            scale=w_sb[:, c : c + 1],
                        )
                        prods.append(p)

                    # DVE: a_i = x_i*w_i + p_{i+4}, then pairwise fold
                    parts = []
                    for c in range(C // 2):
                        a = apool.tile([M, w_t], F32, tag=f"a{c}")
                        nc.vector.scalar_tensor_tensor(
                            a[:],
                            xt[:, c, :],
                            w_sb[:, c : c + 1],
                            prods[c][:],
                            op0=mult,
                            op1=add,
                        )
                        parts.append(a)
                    nc.vector.tensor_add(parts[0][:], parts[0][:], parts[1][:])
                    nc.vector.tensor_add(parts[2][:], parts[2][:], parts[3][:])
                    nc.vector.tensor_add(out_b[:, ts], parts[0][:], parts[2][:])
                # One store per batch: every ax store piece costs ~128 write
                # descriptors (~8 us of SDMA engine-serial time), so fewer
                # pieces steal less bandwidth from the load stream.
                eng = nc.gpsimd if b == 0 else nc.sync
                eng.dma_start(ax[b], out_b[:])

    nc.compile()
    return nc


def kernel(x: np.ndarray, blocks: np.ndarray):
    """Full inputs in, full outputs out. Shards batch across 8 cores."""
    global LAST_RESULTS, _COMPILED
    if _COMPILED is None:
        _COMPILED = _build_module()
    nc = _COMPILED

    x = np.asarray(x, dtype=np.float32)
    blocks_np = np.ascontiguousarray(np.asarray(blocks, dtype=np.float32))
    if x.ndim == 4:
        x = x[:, 0]
    in_maps = [
        {
            "x": np.ascontiguousarray(x[k * B_SH : (k + 1) * B_SH]),
            "blocks": blocks_np,
        }
        for k in range(N_CORES)
    ]
    res = run_bass_kernel_spmd(nc, in_maps, core_ids=list(range(N_CORES)))
    LAST_RESULTS = res
    ax = np.concatenate([res.results[k]["ax"] for k in range(N_CORES)], axis=0)
    a_full = res.results[0]["afull"]
    return ax, a_full


# revision 37
# speedup vs baseline: 1.0278x; 1.0278x over previous
"""Bass/Tile TRN2 kernel for nn_BlockLearnableCompressionMatrix.

Computes, for x (B=16, 1, n=1024, T=2048) f32 and blocks (m=128, c=8) f32:
    w      = tanh(blocks)                          # (128, 8)
    Ax     = einsum('bmct,mc->bmt', x.reshape(B, 128, 8, T), w)   # (16, 128, 2048)
    A_full = block-diagonal expansion of w         # (128, 1024)

Sharding: pure data parallel over batch B across 8 NeuronCores (2 batches
per core); blocks replicated. Each core computes its Ax shard; A_full is
computed redundantly on every core and taken from core 0.

Per-core schedule (memory-bound; ~358 GB/s/core HBM is the roofline — the
kernel moves 19.4 MB/core, so ~54 us of DMA is the floor; measured
~74-77 us incl. ~12 us of framework preamble/barrier):
  - The whole x shard (16 MB = 128 KB/partition) is RESIDENT in one big
    SBUF tile; the 10 load pieces (small first/last for early compute start
    and a short tail) stream back-to-back on the sync HWDGE ring with no
    buffer-recycle waits, sustaining 300-414 GB/s.
  - Per t-chunk, the segment reduce over c=8 splits across engines:
    ACT computes 4 products p_c = x_c * w[:,c] (per-partition scale) into
    PSUM (keeps SBUF ports free for the DMA write stream); DVE does 4
    fused multiply-adds a_i = x_i*w[:,i] + p_{i+4} and 3 pairwise adds,
    the last written into a per-batch SBUF output tile.
  - HBM writes cost ~1 us per descriptor per SDMA engine, so write
    descriptor count dominates every output: Ax goes out as ONE fat store
    per batch (128 x 8 KB rows), and A_full is built as a 16-partition
    image of 8 [w_m | zeros] periods each, so its body writes with just
    16 x 33 KB descriptors (flat A_full is exactly
    [diag_0 | gap_0 | diag_1 | ... | diag_127]).
"""

import os
import sys
import types

import numpy as np

# ---------------------------------------------------------------------------
# NTFF profile hook injection: the trimmed container's `antenv` package lacks
# `axon_hooks`, which bass_utils needs for trace=True under axon. Register the
# ctypes-based hook from trn_agent_boot so profiling works when requested.
# Harmless (and skipped) when unavailable or when tracing is never requested.
try:
    import antenv
    from trn_agent_boot.trn_boot import _ntff_profile_via_ctypes

    if "antenv.axon_hooks" not in sys.modules:
        _hook = _ntff_profile_via_ctypes("/opt/axon/libaxon_pjrt.so")
        _mod = types.ModuleType("antenv.axon_hooks")
        _mod.get_axon_ntff_profile_hook = lambda: _hook
        sys.modules["antenv.axon_hooks"] = _mod
        antenv.axon_hooks = _mod
except Exception:
    pass

import concourse.bass as bass
import concourse.bacc as bacc
import concourse.mybir as mybir
import concourse.tile as tile
from concourse.bass_utils import run_bass_kernel_spmd

N_CORES = 8
B = 16                 # full batch
B_SH = B // N_CORES    # batches per core
M = 128                # number of blocks == partitions
C = 8                  # block size
N_CH = M * C           # 1024 input channels
T = 2048               # time dim
# Asymmetric t-tiling per batch: small first pieces so compute starts early
# (co-queued transfers round-robin on the SDMA engines, so first-piece
# latency grows with queued bytes), small last pieces for a short tail.
TILES_B0 = [64, 448, 512, 512, 512]
TILES_B1 = [512, 512, 512, 384, 128]

F32 = mybir.dt.float32

# Results of the last run (BassKernelResults), for test harnesses that want
# exec_time_ns from a traced run.
LAST_RESULTS = None

_COMPILED = None  # cached bass module


def _build_module():
    mult = mybir.AluOpType.mult
    add = mybir.AluOpType.add
    nc = bacc.Bacc(
        "TRN2",
        target_bir_lowering=False,
        debug=False,
        enable_asserts=False,
        num_devices=N_CORES,
    )
    x = nc.dram_tensor("x", [B_SH, N_CH, T], F32, kind="ExternalInput").ap()
    blocks = nc.dram_tensor("blocks", [M, C], F32, kind="ExternalInput").ap()
    ax = nc.dram_tensor("ax", [B_SH, M, T], F32, kind="ExternalOutput").ap()
    afull = nc.dram_tensor("afull", [M, N_CH], F32, kind="ExternalOutput").ap()

    # x viewed as (b, m, c, t): channel ch = m*8 + c
    xg = x.rearrange("b (m c) t -> b m c t", c=C)

    with tile.TileContext(nc) as tc:
        with (
            tc.tile_pool(name="consts", bufs=1) as cpool,
            tc.tile_pool(name="xin", bufs=1) as xpool,
            tc.tile_pool(name="prod", bufs=2, space="PSUM") as ppool,
            tc.tile_pool(name="acc", bufs=3) as apool,
            tc.tile_pool(name="out", bufs=2) as opool,
        ):
            # --- weights: w = tanh(blocks). Single direct load: one hop
            # through the flooded SDMA engines beats two (a 1-descriptor
            # flat load + local spread each wait ~4-8 us for engine slots
            # once all 8 cores are streaming x).
            blocks_sb = cpool.tile([M, C], F32)
            nc.scalar.dma_start(blocks_sb[:], blocks[:])
            w_sb = cpool.tile([M, C], F32)
            nc.scalar.activation(
                w_sb[:], blocks_sb[:], mybir.ActivationFunctionType.Tanh
            )

            # --- A_full. Flat layout is [diag_0 (8) | gap_0 (1024) | ...
            # | diag_127 (8)]; build an image where each of 16 partitions
            # holds 8 consecutive [w_m | zeros] periods (8256 elems), so the
            # DRAM write needs only 16 fat descriptors (~33 KB each) instead
            # of 128 -- HBM writes cost ~1 us per descriptor per SDMA engine,
            # so descriptor count is the whole cost of this output.
            img = cpool.tile([16, 8 * 1032], F32)
            nc.gpsimd.memset(img[:], 0.0)
            img_diag = img[:].rearrange("p (k q) -> p k q", q=1032)[:, :, 0:C]
            nc.gpsimd.dma_start(img_diag, w_sb[:])
            body = afull.copy()
            _ap = body.ap
            _ap[0] = [8 * 1032, 15]
            _ap[1] = [1, 8 * 1032]
            body.ap = _ap
            nc.gpsimd.dma_start(body, img[0:15, :])
            tail = afull.copy()
            tail.offset = 15 * 8 * 1032
            _ap = tail.ap
            _ap[0] = [7232, 1]
            _ap[1] = [1, 7232]
            tail.ap = _ap
            nc.gpsimd.dma_start(tail, img[15:16, 0:7232])

            # --- main loop over (batch, t-chunk) against the resident x ---
            xbig = xpool.tile([M, C, B_SH * T], F32)
            for b, widths in ((0, TILES_B0), (1, TILES_B1)):
                out_b = opool.tile([M, T], F32, tag="out")
                t0 = 0
                stored = 0
                for w_t in widths:
                    ts = slice(t0, t0 + w_t)
                    bts = slice(b * T + t0, b * T + t0 + w_t)
                    t0 += w_t
                    xt = xbig[:, :, bts]
                    nc.sync.dma_start(xt, xg[b, :, :, ts])

                    # ACT: products for c = 4..7
                    prods = []
                    for c in range(C // 2, C):
                        p = ppool.tile([M, w_t], F32, tag=f"p{c}")
                        nc.scalar.activation(
                            p[:],
                            xt[:, c, :],
                            mybir.ActivationFunctionType.Copy,
                            scale=w_sb[:, c : c + 1],
                        )
                        prods.append(p)

                    # DVE: a_i = x_i*w_i + p_{i+4}, then pairwise fold
                    parts = []
                    for c in range(C // 2):
                        a = apool.tile([M, w_t], F32, tag=f"a{c}")
                        nc.vector.scalar_tensor_tensor(
                            a[:],
                            xt[:, c, :],
                            w_sb[:, c : c + 1],
                            prods[c][:],
                            op0=mult,
                            op1=add,
                        )
                        parts.append(a)
                    nc.vector.tensor_add(parts[0][:], parts[0][:], parts[1][:])
                    nc.vector.tensor_add(parts[2][:], parts[2][:], parts[3][:])
                    nc.vector.tensor_add(out_b[:, ts], parts[0][:], parts[2][:])
                # One store per batch: every ax store piece costs ~128 write
                # descriptors (~8 us of SDMA engine-serial time), so fewer
                # pieces steal less bandwidth from the load stream.
                eng = nc.gpsimd if b == 0 else nc.sync
                eng.dma_start(ax[b], out_b[:])

    nc.compile()
    return nc


def kernel(x: np.ndarray, blocks: np.ndarray):
    """Full inputs in, full outputs out. Shards batch across 8 cores."""
    global LAST_RESULTS, _COMPILED
    if _COMPILED is None:
        _COMPILED = _build_module()
    nc = _COMPILED

    x = np.asarray(x, dtype=np.float32)
    blocks_np = np.ascontiguousarray(np.asarray(blocks, dtype=np.float32))
    if x.ndim == 4:
        x = x[:, 0]
    in_maps = [
        {
            "x": np.ascontiguousarray(x[k * B_SH : (k + 1) * B_SH]),
            "blocks": blocks_np,
        }
        for k in range(N_CORES)
    ]
    res = run_bass_kernel_spmd(nc, in_maps, core_ids=list(range(N_CORES)))
    LAST_RESULTS = res
    ax = np.concatenate([res.results[k]["ax"] for k in range(N_CORES)], axis=0)
    a_full = res.results[0]["afull"]
    return ax, a_full


# revision 38
# speedup vs baseline: 1.1035x; 1.0736x over previous
"""Bass/Tile TRN2 kernel for nn_BlockLearnableCompressionMatrix.

Computes, for x (B=16, 1, n=1024, T=2048) f32 and blocks (m=128, c=8) f32:
    w      = tanh(blocks)                          # (128, 8)
    Ax     = einsum('bmct,mc->bmt', x.reshape(B, 128, 8, T), w)   # (16, 128, 2048)
    A_full = block-diagonal expansion of w         # (128, 1024)

Sharding: pure data parallel over batch B across 8 NeuronCores (2 batches
per core); blocks replicated. Each core computes its Ax shard; A_full is
computed redundantly on every core and taken from core 0.

Per-core schedule (memory-bound; ~358 GB/s/core HBM is the roofline — the
kernel moves 19.4 MB/core, so ~54 us of DMA is the floor; measured
~74-77 us incl. ~12 us of framework preamble/barrier):
  - The whole x shard (16 MB = 128 KB/partition) is RESIDENT in one big
    SBUF tile; the 10 load pieces (small first/last for early compute start
    and a short tail) stream back-to-back on the sync HWDGE ring with no
    buffer-recycle waits, sustaining 300-414 GB/s.
  - Per t-chunk, the segment reduce over c=8 splits across engines:
    ACT computes 4 products p_c = x_c * w[:,c] (per-partition scale) into
    PSUM (keeps SBUF ports free for the DMA write stream); DVE does 4
    fused multiply-adds a_i = x_i*w[:,i] + p_{i+4} and 3 pairwise adds,
    the last written into a per-batch SBUF output tile.
  - HBM writes cost ~1 us per descriptor per SDMA engine, so write
    descriptor count dominates every output: Ax goes out as ONE fat store
    per batch (128 x 8 KB rows), and A_full is built as a 16-partition
    image of 8 [w_m | zeros] periods each, so its body writes with just
    16 x 33 KB descriptors (flat A_full is exactly
    [diag_0 | gap_0 | diag_1 | ... | diag_127]).
"""

import os
import sys
import types

import numpy as np

# ---------------------------------------------------------------------------
# NTFF profile hook injection: the trimmed container's `antenv` package lacks
# `axon_hooks`, which bass_utils needs for trace=True under axon. Register the
# ctypes-based hook from trn_agent_boot so profiling works when requested.
# Harmless (and skipped) when unavailable or when tracing is never requested.
try:
    import antenv
    from trn_agent_boot.trn_boot import _ntff_profile_via_ctypes

    if "antenv.axon_hooks" not in sys.modules:
        _hook = _ntff_profile_via_ctypes("/opt/axon/libaxon_pjrt.so")
        _mod = types.ModuleType("antenv.axon_hooks")
        _mod.get_axon_ntff_profile_hook = lambda: _hook
        sys.modules["antenv.axon_hooks"] = _mod
        antenv.axon_hooks = _mod
except Exception:
    pass

import concourse.bass as bass
import concourse.bacc as bacc
import concourse.mybir as mybir
import concourse.tile as tile
from concourse.bass_utils import run_bass_kernel_spmd

N_CORES = 8
B = 16                 # full batch
B_SH = B // N_CORES    # batches per core
M = 128                # number of blocks == partitions
C = 8                  # block size
N_CH = M * C           # 1024 input channels
T = 2048               # time dim
# Asymmetric t-tiling per batch: small first pieces so compute starts early
# (co-queued transfers round-robin on the SDMA engines, so first-piece
# latency grows with queued bytes), small last pieces for a short tail.
TILES_B0 = [64, 448, 512, 512, 512]
TILES_B1 = [512, 512, 512, 448, 64]

F32 = mybir.dt.float32

# Results of the last run (BassKernelResults), for test harnesses that want
# exec_time_ns from a traced run.
LAST_RESULTS = None

_COMPILED = None  # cached bass module


def _build_module():
    mult = mybir.AluOpType.mult
    add = mybir.AluOpType.add
    nc = bacc.Bacc(
        "TRN2",
        target_bir_lowering=False,
        debug=False,
        enable_asserts=False,
        num_devices=N_CORES,
    )
    x = nc.dram_tensor("x", [B_SH, N_CH, T], F32, kind="ExternalInput").ap()
    blocks = nc.dram_tensor("blocks", [M, C], F32, kind="ExternalInput").ap()
    ax = nc.dram_tensor("ax", [B_SH, M, T], F32, kind="ExternalOutput").ap()
    afull = nc.dram_tensor("afull", [M, N_CH], F32, kind="ExternalOutput").ap()

    # x viewed as (b, m, c, t): channel ch = m*8 + c
    xg = x.rearrange("b (m c) t -> b m c t", c=C)

    with tile.TileContext(nc) as tc:
        with (
            tc.tile_pool(name="consts", bufs=1) as cpool,
            tc.tile_pool(name="xin", bufs=1) as xpool,
            tc.tile_pool(name="prod", bufs=2, space="PSUM") as ppool,
            tc.tile_pool(name="acc", bufs=3) as apool,
            tc.tile_pool(name="out", bufs=2) as opool,
        ):
            # --- weights: w = tanh(blocks). Single direct load: one hop
            # through the flooded SDMA engines beats two (a 1-descriptor
            # flat load + local spread each wait ~4-8 us for engine slots
            # once all 8 cores are streaming x).
            blocks_sb = cpool.tile([M, C], F32)
            nc.scalar.dma_start(blocks_sb[:], blocks[:])
            w_sb = cpool.tile([M, C], F32)
            nc.scalar.activation(
                w_sb[:], blocks_sb[:], mybir.ActivationFunctionType.Tanh
            )

            # --- A_full. Flat layout is [diag_0 (8) | gap_0 (1024) | ...
            # | diag_127 (8)]; build an image where each of 16 partitions
            # holds 8 consecutive [w_m | zeros] periods (8256 elems), so the
            # DRAM write needs only 16 fat descriptors (~33 KB each) instead
            # of 128 -- HBM writes cost ~1 us per descriptor per SDMA engine,
            # so descriptor count is the whole cost of this output.
            img = cpool.tile([16, 8 * 1032], F32)
            nc.gpsimd.memset(img[:], 0.0)
            img_diag = img[:].rearrange("p (k q) -> p k q", q=1032)[:, :, 0:C]
            nc.gpsimd.dma_start(img_diag, w_sb[:])
            body = afull.copy()
            _ap = body.ap
            _ap[0] = [8 * 1032, 15]
            _ap[1] = [1, 8 * 1032]
            body.ap = _ap
            nc.gpsimd.dma_start(body, img[0:15, :])
            tail = afull.copy()
            tail.offset = 15 * 8 * 1032
            _ap = tail.ap
            _ap[0] = [7232, 1]
            _ap[1] = [1, 7232]
            tail.ap = _ap
            nc.gpsimd.dma_start(tail, img[15:16, 0:7232])

            # --- main loop over (batch, t-chunk) against the resident x ---
            xbig = xpool.tile([M, C, B_SH * T], F32)
            for b, widths in ((0, TILES_B0), (1, TILES_B1)):
                out_b = opool.tile([M, T], F32, tag="out")
                t0 = 0
                stored = 0
                for w_t in widths:
                    ts = slice(t0, t0 + w_t)
                    bts = slice(b * T + t0, b * T + t0 + w_t)
                    t0 += w_t
                    xt = xbig[:, :, bts]
                    nc.sync.dma_start(xt, xg[b, :, :, ts])

                    # ACT: products for c = 4..7
                    prods = []
                    for c in range(C // 2, C):
                        p = ppool.tile([M, w_t], F32, tag=f"p{c}")
                        nc.scalar.activation(
                            p[:],
                            xt[:, c, :],
                            mybir.ActivationFunctionType.Copy,
                            scale=w_sb[:, c : c + 1],
                        )
                        prods.append(p)

                    # DVE: a_i = x_i*w_i + p_{i+4}, then pairwise fold
                    parts = []
                    for c in range(C // 2):
                        a = apool.tile([M, w_t], F32, tag=f"a{c}")
                        nc.vector.scalar_tensor_tensor(
                            a[:],
                            xt[:, c, :],
                            w_sb[:, c : c + 1],
                            prods[c][:],
                            op0=mult,
                            op1=add,
                        )
                        parts.append(a)
                    nc.vector.tensor_add(parts[0][:], parts[0][:], parts[1][:])
                    nc.vector.tensor_add(parts[2][:], parts[2][:], parts[3][:])
                    nc.vector.tensor_add(out_b[:, ts], parts[0][:], parts[2][:])
                # One store per batch: every ax store piece costs ~128 write
                # descriptors (~8 us of SDMA engine-serial time), so fewer
                # pieces steal less bandwidth from the load stream.
                eng = nc.gpsimd if b == 0 else nc.sync
                eng.dma_start(ax[b], out_b[:])

    nc.compile()
    return nc


def kernel(x: np.ndarray, blocks: np.ndarray):
    """Full inputs in, full outputs out. Shards batch across 8 cores."""
    global LAST_RESULTS, _COMPILED
    if _COMPILED is None:
        _COMPILED = _build_module()
    nc = _COMPILED

    x = np.asarray(x, dtype=np.float32)
    blocks_np = np.ascontiguousarray(np.asarray(blocks, dtype=np.float32))
    if x.ndim == 4:
        x = x[:, 0]
    in_maps = [
        {
            "x": np.ascontiguousarray(x[k * B_SH : (k + 1) * B_SH]),
            "blocks": blocks_np,
        }
        for k in range(N_CORES)
    ]
    res = run_bass_kernel_spmd(nc, in_maps, core_ids=list(range(N_CORES)))
    LAST_RESULTS = res
    ax = np.concatenate([res.results[k]["ax"] for k in range(N_CORES)], axis=0)
    a_full = res.results[0]["afull"]
    return ax, a_full
